# revision 30
# baseline (speedup 1.0000x reference)
"""Contrastive loss kernel for Trainium2 (8 NeuronCores).

Strategy: 2-D shard of the pairwise score computation (refining the
caption-axis sharding hint): 4-way split of the caption axis x 2-way split
of the image axis.  Each core computes its [16 captions * 50 words,
32 images * 36 regions] block of the raw pairwise dot tensor
g[i,w,j,r] = s[i,w,:] . im[j,r,:]  (the dominant 15 GFLOP contraction over
D=1024) on the TensorEngine in fp8-e4m3 with DoubleRow perf mode (2 fp8
weights per PE cell, 256-deep contraction per matmul) and fp32 PSUM
accumulation.  The 2-D shard minimizes per-core HBM traffic (s block +
im half ~ 2.0 MB vs 2.8 MB for a 1-D split).  g is written back as
fp8-e4m3 (|g| <= ~170 < 240 max).  The remaining cheap reductions
(leaky-relu attention, softmax, top-k word pooling, entity-matched direct
score, margin reduction) run on host in float32; fp8 quantization of the
g contraction shifts the final scalar by ~1e-3 relative, far inside the
2e-2 tolerance.
"""

import os
import sys

import numpy as np
import ml_dtypes

sys.path.insert(0, "/opt/trn_rl_repo")

B, R, L, D = 64, 36, 50, 1024
N_CORES = 8
CA, CB = 4, 2                  # caption-axis x image-axis core grid
BT_LOC = B // CA               # 16 captions per core
BI_LOC = B // CB               # 32 images per core
M = BT_LOC * L                 # 800 caption-words per core (moving dim)
MH = M // 2                    # 400-wide m-halves (PSUM free-dim limit)
N = BI_LOC * R                 # 1152 image-regions per core = 9 n-tiles
K = D                          # 1024 contraction
NT = N // 128                  # 9
KQ = K // 256                  # 4 DoubleRow k-pair tiles
LAMBDA_SOFTMAX = 9.0
MARGIN = 0.2
EPS = 1e-8

# im columns are DMA'd per k-pair in two chunks: tiles 0-3 and tiles 4-8
CHUNKS = [(0, 4), (4, 5)]      # (first tile, tile count)
# compute groups of <=2 n-tiles (4 PSUM banks each, 2 groups in flight)
GROUPS = [(0, 2), (2, 2), (4, 2), (6, 2), (8, 1)]
# output DMA groups (consecutive tiles shipped in one DMA) + lane each;
# the final tiles ship as singles on three different lanes so the closing
# chain never queues behind another out-DMA
OUT_GROUPS = [(0, 1), (2, 3), (4, 5), (6, 7), (8,)]
OUT_LANES = ["av", "sp", "av", "av", "sp"]
# psum->sbuf cast copies alternate between the DVE and Activation engines
COPY_ENGINES = ["dv", "av"]

_CACHE = {}
LAST_RESULTS = None  # BassKernelResults from the most recent run (for test.py)

# Input DMA schedule: ordered ops, lane "sp" -> HWDGE via nc.sync,
# "pl" -> SWDGE on the Pool engine via nc.gpsimd.
#   ("st", (q0, q1), lane): sT k-pairs [q0, q1) in one DMA
#   ("it", q, ci, lane):    im chunk ci of k-pair q
SCHEDULE = [
    ("it", 0, 0, "sp"),
    ("st", (0, 1), "sp"),
    ("it", 1, 0, "pl"),
    ("st", (1, 2), "sp"),
    ("it", 2, 0, "pl"),
    ("st", (2, 3), "sp"),
    ("it", 3, 0, "pl"),
    ("st", (3, 4), "sp"),
    ("it", 0, 1, "pl"),
    ("it", 1, 1, "sp"),
    ("it", 2, 1, "pl"),
    ("it", 3, 1, "sp"),
]


def _build_bass():
    import concourse.bacc as bacc
    import concourse.mybir as mybir
    import concourse.tile as tile

    nc = bacc.Bacc(
        "TRN2",
        target_bir_lowering=False,
        debug=False,
        enable_asserts=False,
        num_devices=N_CORES,
    )
    f32 = mybir.dt.float32
    fp8 = mybir.dt.float8e4
    sT = nc.dram_tensor("sT", [K, M], fp8, kind="ExternalInput").ap()
    imT = nc.dram_tensor("imT", [K, N], fp8, kind="ExternalInput").ap()
    gT = nc.dram_tensor("gT", [N, M], fp8, kind="ExternalOutput").ap()

    with tile.TileContext(nc) as tc:
        with (
            tc.tile_pool(name="imt", bufs=KQ * len(CHUNKS)) as ip,
            tc.tile_pool(name="st", bufs=KQ + 1) as sp,
            tc.tile_pool(name="ps", bufs=8, space="PSUM") as pp,
            tc.tile_pool(name="out", bufs=4) as outp,
        ):
            sts = []
            its = [[None] * len(CHUNKS) for _ in range(KQ)]

            def dma_in_pair(eng, dst, src_rows, col0, width, half=None):
                # dst sbuf [128, 2*width]; src dram rows [q*256, q*256+256)
                d = dst[:].rearrange("p (i x) -> p i x", i=2)
                s = src_rows[:, col0:col0 + width].rearrange(
                    "(i p) x -> p i x", i=2)
                if half is not None:
                    h0 = half * (width // 2)
                    d = d[:, :, h0:h0 + width // 2]
                    s = s[:, :, h0:h0 + width // 2]
                eng.dma_start(d, s)

            # Two parallel descriptor-generation lanes: nc.sync -> HWDGE
            # (shared serial device, ~625ns/DMA) and nc.gpsimd -> SWDGE on
            # the otherwise-idle Pool engine (~1.1us/DMA).  The first matmul
            # needs it(q0,c0) + the first st half, so those go first on the
            # fast lane; st for q1-3 is one merged 4D-AP DMA (one HWDGE slot
            # for 3 k-pairs); im chunks stream q-ordered so the last-arriving
            # chunk gates only the final q3 passes.
            lanes = {"sp": nc.sync, "pl": nc.gpsimd,
                     "dv": nc.vector, "av": nc.scalar}
            st_tiles = {}  # q -> (tile, nq, q_off)
            for op_i, op in enumerate(SCHEDULE):
                if op[0] == "st":
                    (q0, q1), lane = op[1], op[2]
                    nq = q1 - q0
                    st = sp.tile([128, nq * 2 * M], fp8, tag="st",
                                 name=f"st_{q0}")
                    lanes[lane].dma_start(
                        st[:].rearrange("p (q i m) -> p q i m", q=nq, i=2),
                        sT[q0 * 256:q1 * 256, :].rearrange(
                            "(q i p) m -> p q i m", q=nq, i=2),
                    )
                    for q in range(q0, q1):
                        st_tiles[q] = (st, nq, q - q0)
                else:
                    _, q, ci, lane = op
                    t0c, tnc = CHUNKS[ci]
                    it = ip.tile([128, 2 * tnc * 128], fp8, tag="imt",
                                 name=f"it_{q}_{ci}")
                    dma_in_pair(lanes[lane], it,
                                imT[q * 256:(q + 1) * 256, :],
                                t0c * 128, tnc * 128)
                    its[q][ci] = it

            def st_slice(q, h):
                st, nq, qo = st_tiles[q]
                v = st[:].rearrange("p (q i m) -> p q i m", q=nq, i=2)
                return v[:, qo, :, h * MH:(h + 1) * MH]

            DR = mybir.MatmulPerfMode.DoubleRow

            def it_slice(q, t):
                for ci, (t0, tn) in enumerate(CHUNKS):
                    if t0 <= t < t0 + tn:
                        w = tn * 128
                        x0 = (t - t0) * 128
                        return its[q][ci][:].rearrange(
                            "p (i x) -> p i x", i=2)[:, :, x0:x0 + 128]
                raise AssertionError

            tile_to_og = {}
            for ogi, og in enumerate(OUT_GROUPS):
                for t in og:
                    tile_to_og[t] = ogi
            og_tiles = {}   # ogi -> out sbuf tile

            def copy_ps(ce_name, dst, src):
                ce = lanes[ce_name]
                if ce is nc.scalar:
                    ce.copy(dst, src)
                else:
                    ce.tensor_copy(dst, src)

            for gi, (g0, gn) in enumerate(GROUPS):
                tiles = [(g0 + j, h) for j in range(gn) for h in range(2)]
                pss = {}
                for th in tiles:
                    ps = pp.tile([128, MH], f32, tag="ps", name=f"ps_{th[0]}_{th[1]}")
                    pss[th] = ps
                for q in range(KQ):
                    for (t, h) in tiles:
                        nc.tensor.matmul(
                            pss[(t, h)][:, :],
                            it_slice(q, t),
                            st_slice(q, h),
                            start=(q == 0),
                            stop=(q == KQ - 1),
                            perf_mode=DR,
                        )
                for j in range(gn):
                    t = g0 + j
                    ogi = tile_to_og[t]
                    og = OUT_GROUPS[ogi]
                    if ogi not in og_tiles:
                        og_tiles[ogi] = outp.tile(
                            [128, len(og) * M], fp8, tag="out",
                            name=f"out_{ogi}")
                    ot = og_tiles[ogi]
                    o0 = og.index(t) * M
                    for h in range(2):
                        copy_ps(COPY_ENGINES[(2 * t + h) % len(COPY_ENGINES)],
                                ot[:, o0 + h * MH:o0 + (h + 1) * MH],
                                pss[(t, h)][:, :])
                    if t == og[-1]:
                        ng = len(og)
                        dst = gT[og[0] * 128:(og[0] + ng) * 128, :]
                        if ng > 1:
                            dst = dst.rearrange("(b p) m -> p b m", b=ng)
                        lanes[OUT_LANES[ogi]].dma_start(dst, ot[:])
    nc.compile()
    return nc


def _run_device(s_np, im_np):
    """Returns g4 [B, B, L, R]: g4[i,j,w,r] = s[i,w] . im[j,r]."""
    global LAST_RESULTS
    from concourse import bass_utils

    if "nc" not in _CACHE:
        _CACHE["nc"] = _build_bass()
    nc = _CACHE["nc"]

    fp8 = ml_dtypes.float8_e4m3
    imq = im_np.reshape(B * R, K).astype(fp8)                   # [2304, 1024]
    sq = s_np.reshape(B * L, K).astype(fp8)                     # [3200, 1024]
    in_maps = []
    for c in range(N_CORES):
        ca, cb = divmod(c, CB)
        sblk = sq[ca * M:(ca + 1) * M]                          # [800, 1024]
        iblk = imq[cb * N:(cb + 1) * N]                         # [1152, 1024]
        in_maps.append({
            "sT": np.ascontiguousarray(sblk.T),                 # [1024, 800]
            "imT": np.ascontiguousarray(iblk.T),                # [1024, 1152]
        })
    res = bass_utils.run_bass_kernel_spmd(
        nc, in_maps, core_ids=list(range(N_CORES)),
        trace=bool(os.environ.get("KERNEL_TRACE")),
    )
    LAST_RESULTS = res
    g4 = np.empty((B, B, L, R), dtype=np.float32)
    for c in range(N_CORES):
        ca, cb = divmod(c, CB)
        gb = np.asarray(res.results[c]["gT"], dtype=np.float32)  # [1152, 800]
        g4[ca * BT_LOC:(ca + 1) * BT_LOC,
           cb * BI_LOC:(cb + 1) * BI_LOC] = (
            gb.reshape(BI_LOC, R, BT_LOC, L).transpose(2, 0, 3, 1))
    return g4


def _host_finish(g4, im, s, img_ent, cap_ent, cap_lens):
    f32 = np.float32
    w_idx = np.arange(L)
    word_valid = w_idx[None, :] < cap_lens[:, None]             # [Bt, L]

    attn = np.where(g4 > 0, g4, f32(0.1) * g4)
    attn = attn * word_valid[:, None, :, None].astype(f32)
    attn = attn / (np.sqrt(np.sum(attn * attn, axis=2, keepdims=True)) + f32(EPS))
    z = attn * f32(LAMBDA_SOFTMAX)
    z = z - z.max(axis=-1, keepdims=True)
    e = np.exp(z)
    a = e / e.sum(axis=-1, keepdims=True)
    a = a * (a > 1.0 / R).astype(f32)

    dot_swc = np.sum(a * g4, axis=-1)                           # [Bt,Bi,L]
    gram = np.einsum("jrd,jqd->jrq", im, im)                    # [Bi,R,R]
    t = np.einsum("ijwr,jrq->ijwq", a, gram, optimize=True)
    wc_sq = np.sum(t * a, axis=-1)
    wc_norm = np.sqrt(np.maximum(wc_sq, f32(1e-24)))
    ns = np.sqrt(np.sum(s * s, axis=-1))                        # [Bt,L]
    cos = dot_swc / np.maximum(ns[:, None, :] * wc_norm, f32(EPS))
    cos = np.where(word_valid[:, None, :], cos, f32(-np.inf))
    srt = np.sort(cos, axis=-1)[..., ::-1]
    k = cap_lens - cap_lens // 3
    keep = w_idx[None, None, :] < k[:, None, None]
    latent = np.where(keep, srt, f32(0.0)).sum(axis=-1) / k[:, None].astype(f32)

    n_min = np.minimum(cap_lens, 50)
    ent_ok = (cap_ent != 0) & (w_idx[None, :] < n_min[:, None])
    match = (cap_ent[:, None, :, None] == img_ent[None, :, None, :]) \
        & ent_ok[:, None, :, None]
    nim = np.sqrt(np.sum(im * im, axis=-1))                     # [Bi,R]
    denom = np.maximum(ns[:, None, :, None] * nim[None, :, None, :], f32(EPS))
    direct = np.where(match, g4 / denom, f32(0.0)).sum(axis=(2, 3)) \
        / n_min[:, None].astype(f32)

    scores = latent + direct                                    # [Bt,Bi]
    diag = np.diag(scores).copy()
    cost_s = np.maximum(f32(MARGIN) + scores - diag[:, None], f32(0.0))
    cost_im = np.maximum(f32(MARGIN) + scores - diag[None, :], f32(0.0))
    np.fill_diagonal(cost_s, 0.0)
    np.fill_diagonal(cost_im, 0.0)
    return np.float32(cost_s.max(axis=1).sum() + cost_im.max(axis=0).sum())


def kernel(im, s, image_entity_idxs, caps_entity_idxs, s_l):
    im = np.asarray(im, dtype=np.float32)
    s = np.asarray(s, dtype=np.float32)
    img_ent = np.asarray(image_entity_idxs)
    cap_ent = np.asarray(caps_entity_idxs)
    cap_lens = np.asarray(s_l)
    g4 = _run_device(s, im)
    return _host_finish(g4, im, s, img_ent, cap_ent, cap_lens)


# revision 41
# speedup vs baseline: 1.0124x; 1.0124x over previous
"""Contrastive loss kernel for Trainium2 (8 NeuronCores).

Strategy: 2-D shard of the pairwise score computation (refining the
caption-axis sharding hint): 4-way split of the caption axis x 2-way split
of the image axis.  Each core computes its [16 captions * 50 words,
32 images * 36 regions] block of the raw pairwise dot tensor
g[i,w,j,r] = s[i,w,:] . im[j,r,:]  (the dominant 15 GFLOP contraction over
D=1024) on the TensorEngine in fp8-e4m3 with DoubleRow perf mode (2 fp8
weights per PE cell, 256-deep contraction per matmul) and fp32 PSUM
accumulation.  The 2-D shard minimizes per-core HBM traffic (s block +
im half ~ 2.0 MB vs 2.8 MB for a 1-D split).  g is written back as
fp8-e4m3 (|g| <= ~170 < 240 max).  The remaining cheap reductions
(leaky-relu attention, softmax, top-k word pooling, entity-matched direct
score, margin reduction) run on host in float32; fp8 quantization of the
g contraction shifts the final scalar by ~1e-3 relative, far inside the
2e-2 tolerance.
"""

import os
import sys

import numpy as np
import ml_dtypes

sys.path.insert(0, "/opt/trn_rl_repo")

B, R, L, D = 64, 36, 50, 1024
N_CORES = 8
CA, CB = 4, 2                  # caption-axis x image-axis core grid
BT_LOC = B // CA               # 16 captions per core
BI_LOC = B // CB               # 32 images per core
M = BT_LOC * L                 # 800 caption-words per core (moving dim)
MH = M // 2                    # 400-wide m-halves (PSUM free-dim limit)
N = BI_LOC * R                 # 1152 image-regions per core = 9 n-tiles
K = D                          # 1024 contraction
NT = N // 128                  # 9
KQ = K // 256                  # 4 DoubleRow k-pair tiles
LAMBDA_SOFTMAX = 9.0
MARGIN = 0.2
EPS = 1e-8

# im columns are DMA'd per k-pair in two chunks: tiles 0-3 and tiles 4-8
CHUNKS = [(0, 4), (4, 5)]      # (first tile, tile count)
# compute groups of <=2 n-tiles (4 PSUM banks each, 2 groups in flight)
GROUPS = [(0, 2), (2, 2), (4, 2), (6, 2), (8, 1)]
# output DMA groups (consecutive tiles shipped in one DMA) + lane each;
# the final tiles ship as singles on three different lanes so the closing
# chain never queues behind another out-DMA
OUT_GROUPS = [(0, 1), (2, 3), (4, 5), (6, 7), (8,)]
OUT_LANES = ["av", "sp", "av", "av", "sp"]
# psum->sbuf cast copies alternate between the DVE and Activation engines
COPY_ENGINES = ["dv", "av"]

_CACHE = {}
LAST_RESULTS = None  # BassKernelResults from the most recent run (for test.py)

# Input DMA schedule: ordered ops, lane "sp" -> HWDGE via nc.sync,
# "pl" -> SWDGE on the Pool engine via nc.gpsimd.
# The host packs sT and imT row-wise into one DRAM tensor
# packed[k, :] = [sT[k, 0:800] | imT[k, 0:1152]], so one DMA fetches a
# k-pair's sT AND its first im chunk together (columns 0:1312):
#   ("pq", q, lane): packed rows of k-pair q, cols [0, 1312)  (st + im c0)
#   ("c1", q, lane): packed rows of k-pair q, cols [1312, 1952) (im c1)
SCHEDULE = [
    ("c0", 0, "sp"),
    ("st", 0, "sp"),
    ("c0", 1, "pl"),
    ("st", 1, "sp"),
    ("c0", 2, "pl"),
    ("st", 2, "sp"),
    ("c0", 3, "pl"),
    ("st", 3, "sp"),
    ("c1", 0, "sp"),
    ("c1", 1, "sp"),
    ("c1", 2, "sp"),
    ("c1", 3, "sp"),
]
PQW = M + 4 * 128              # 1312 packed st+c0 columns
C0W = 4 * 128                  # 512 c0 columns (packed cols M:PQW)
C1W = 5 * 128                  # 640 c1 columns (packed cols PQW:)


def _build_bass():
    import concourse.bacc as bacc
    import concourse.mybir as mybir
    import concourse.tile as tile

    nc = bacc.Bacc(
        "TRN2",
        target_bir_lowering=False,
        debug=False,
        enable_asserts=False,
        num_devices=N_CORES,
    )
    f32 = mybir.dt.float32
    fp8 = mybir.dt.float8e4
    packed = nc.dram_tensor("packed", [K, M + N], fp8,
                            kind="ExternalInput").ap()
    gT = nc.dram_tensor("gT", [N, M], fp8, kind="ExternalOutput").ap()

    with tile.TileContext(nc) as tc:
        with (
            tc.tile_pool(name="pq", bufs=2 * KQ) as qp,
            tc.tile_pool(name="c1", bufs=KQ) as cp,
            tc.tile_pool(name="ps", bufs=8, space="PSUM") as pp,
            tc.tile_pool(name="out", bufs=4) as outp,
        ):
            # Two parallel descriptor-generation lanes: nc.sync -> HWDGE
            # (shared serial device, ~625ns/DMA) and nc.gpsimd -> SWDGE on
            # the otherwise-idle Pool engine (~1.1us/DMA).  Each k-pair's
            # sT + first im chunk land in ONE transfer (packed layout), so
            # a q-wave of matmuls unlocks atomically per arrival.
            lanes = {"sp": nc.sync, "pl": nc.gpsimd,
                     "dv": nc.vector, "av": nc.scalar}
            # per-q views: (tile, base column) for the sT, im-c0, im-c1 data
            stv = [None] * KQ
            c0v = [None] * KQ
            c1v = [None] * KQ
            spans = {"pq": (0, PQW), "st": (0, M),
                     "c0": (M, C0W), "c1": (PQW, C1W)}
            for op in SCHEDULE:
                kind, q, lane = op
                col0, width = spans[kind]
                pool = qp if kind in ("pq", "st", "c0") else cp
                t_ = pool.tile([128, 2 * width], fp8, tag=kind,
                               name=f"{kind}_{q}")
                lanes[lane].dma_start(
                    t_[:].rearrange("p (i x) -> p i x", i=2),
                    packed[q * 256:(q + 1) * 256,
                           col0:col0 + width].rearrange(
                        "(i p) x -> p i x", i=2),
                )
                if kind == "pq":
                    stv[q] = (t_, 0)
                    c0v[q] = (t_, M)
                elif kind == "st":
                    stv[q] = (t_, 0)
                elif kind == "c0":
                    c0v[q] = (t_, 0)
                else:
                    c1v[q] = (t_, 0)

            def st_slice(q, h):
                t_, base = stv[q]
                return t_[:].rearrange("p (i x) -> p i x", i=2)[
                    :, :, base + h * MH:base + (h + 1) * MH]

            DR = mybir.MatmulPerfMode.DoubleRow

            def it_slice(q, t):
                if t < 4:
                    t_, base = c0v[q]
                    x0 = base + t * 128
                else:
                    t_, base = c1v[q]
                    x0 = base + (t - 4) * 128
                return t_[:].rearrange("p (i x) -> p i x", i=2)[
                    :, :, x0:x0 + 128]

            tile_to_og = {}
            for ogi, og in enumerate(OUT_GROUPS):
                for t in og:
                    tile_to_og[t] = ogi
            og_tiles = {}   # ogi -> out sbuf tile

            def copy_ps(ce_name, dst, src):
                ce = lanes[ce_name]
                if ce is nc.scalar:
                    ce.copy(dst, src)
                else:
                    ce.tensor_copy(dst, src)

            for gi, (g0, gn) in enumerate(GROUPS):
                tiles = [(g0 + j, h) for j in range(gn) for h in range(2)]
                pss = {}
                for th in tiles:
                    ps = pp.tile([128, MH], f32, tag="ps", name=f"ps_{th[0]}_{th[1]}")
                    pss[th] = ps
                for q in range(KQ):
                    for (t, h) in tiles:
                        nc.tensor.matmul(
                            pss[(t, h)][:, :],
                            it_slice(q, t),
                            st_slice(q, h),
                            start=(q == 0),
                            stop=(q == KQ - 1),
                            perf_mode=DR,
                        )
                for j in range(gn):
                    t = g0 + j
                    ogi = tile_to_og[t]
                    og = OUT_GROUPS[ogi]
                    if ogi not in og_tiles:
                        og_tiles[ogi] = outp.tile(
                            [128, len(og) * M], fp8, tag="out",
                            name=f"out_{ogi}")
                    ot = og_tiles[ogi]
                    o0 = og.index(t) * M
                    for h in range(2):
                        copy_ps(COPY_ENGINES[(2 * t + h) % len(COPY_ENGINES)],
                                ot[:, o0 + h * MH:o0 + (h + 1) * MH],
                                pss[(t, h)][:, :])
                    if t == og[-1]:
                        ng = len(og)
                        dst = gT[og[0] * 128:(og[0] + ng) * 128, :]
                        if ng > 1:
                            dst = dst.rearrange("(b p) m -> p b m", b=ng)
                        lanes[OUT_LANES[ogi]].dma_start(dst, ot[:])
    nc.compile()
    return nc


def _run_device(s_np, im_np):
    """Returns g4 [B, B, L, R]: g4[i,j,w,r] = s[i,w] . im[j,r]."""
    global LAST_RESULTS
    from concourse import bass_utils

    if "nc" not in _CACHE:
        _CACHE["nc"] = _build_bass()
    nc = _CACHE["nc"]

    fp8 = ml_dtypes.float8_e4m3
    imq = im_np.reshape(B * R, K).astype(fp8)                   # [2304, 1024]
    sq = s_np.reshape(B * L, K).astype(fp8)                     # [3200, 1024]
    in_maps = []
    for c in range(N_CORES):
        ca, cb = divmod(c, CB)
        sblk = sq[ca * M:(ca + 1) * M]                          # [800, 1024]
        iblk = imq[cb * N:(cb + 1) * N]                         # [1152, 1024]
        pk = np.empty((K, M + N), dtype=fp8)
        pk[:, 0:M] = sblk.T
        pk[:, M:M + N] = iblk.T
        in_maps.append({"packed": pk})
    res = bass_utils.run_bass_kernel_spmd(
        nc, in_maps, core_ids=list(range(N_CORES)),
        trace=bool(os.environ.get("KERNEL_TRACE")),
    )
    LAST_RESULTS = res
    g4 = np.empty((B, B, L, R), dtype=np.float32)
    for c in range(N_CORES):
        ca, cb = divmod(c, CB)
        gb = np.asarray(res.results[c]["gT"], dtype=np.float32)  # [1152, 800]
        g4[ca * BT_LOC:(ca + 1) * BT_LOC,
           cb * BI_LOC:(cb + 1) * BI_LOC] = (
            gb.reshape(BI_LOC, R, BT_LOC, L).transpose(2, 0, 3, 1))
    return g4


def _host_finish(g4, im, s, img_ent, cap_ent, cap_lens):
    f32 = np.float32
    w_idx = np.arange(L)
    word_valid = w_idx[None, :] < cap_lens[:, None]             # [Bt, L]

    attn = np.where(g4 > 0, g4, f32(0.1) * g4)
    attn = attn * word_valid[:, None, :, None].astype(f32)
    attn = attn / (np.sqrt(np.sum(attn * attn, axis=2, keepdims=True)) + f32(EPS))
    z = attn * f32(LAMBDA_SOFTMAX)
    z = z - z.max(axis=-1, keepdims=True)
    e = np.exp(z)
    a = e / e.sum(axis=-1, keepdims=True)
    a = a * (a > 1.0 / R).astype(f32)

    dot_swc = np.sum(a * g4, axis=-1)                           # [Bt,Bi,L]
    gram = np.einsum("jrd,jqd->jrq", im, im)                    # [Bi,R,R]
    t = np.einsum("ijwr,jrq->ijwq", a, gram, optimize=True)
    wc_sq = np.sum(t * a, axis=-1)
    wc_norm = np.sqrt(np.maximum(wc_sq, f32(1e-24)))
    ns = np.sqrt(np.sum(s * s, axis=-1))                        # [Bt,L]
    cos = dot_swc / np.maximum(ns[:, None, :] * wc_norm, f32(EPS))
    cos = np.where(word_valid[:, None, :], cos, f32(-np.inf))
    srt = np.sort(cos, axis=-1)[..., ::-1]
    k = cap_lens - cap_lens // 3
    keep = w_idx[None, None, :] < k[:, None, None]
    latent = np.where(keep, srt, f32(0.0)).sum(axis=-1) / k[:, None].astype(f32)

    n_min = np.minimum(cap_lens, 50)
    ent_ok = (cap_ent != 0) & (w_idx[None, :] < n_min[:, None])
    match = (cap_ent[:, None, :, None] == img_ent[None, :, None, :]) \
        & ent_ok[:, None, :, None]
    nim = np.sqrt(np.sum(im * im, axis=-1))                     # [Bi,R]
    denom = np.maximum(ns[:, None, :, None] * nim[None, :, None, :], f32(EPS))
    direct = np.where(match, g4 / denom, f32(0.0)).sum(axis=(2, 3)) \
        / n_min[:, None].astype(f32)

    scores = latent + direct                                    # [Bt,Bi]
    diag = np.diag(scores).copy()
    cost_s = np.maximum(f32(MARGIN) + scores - diag[:, None], f32(0.0))
    cost_im = np.maximum(f32(MARGIN) + scores - diag[None, :], f32(0.0))
    np.fill_diagonal(cost_s, 0.0)
    np.fill_diagonal(cost_im, 0.0)
    return np.float32(cost_s.max(axis=1).sum() + cost_im.max(axis=0).sum())


def kernel(im, s, image_entity_idxs, caps_entity_idxs, s_l):
    im = np.asarray(im, dtype=np.float32)
    s = np.asarray(s, dtype=np.float32)
    img_ent = np.asarray(image_entity_idxs)
    cap_ent = np.asarray(caps_entity_idxs)
    cap_lens = np.asarray(s_l)
    g4 = _run_device(s, im)
    return _host_finish(g4, im, s, img_ent, cap_ent, cap_lens)


# revision 45
# speedup vs baseline: 1.0156x; 1.0031x over previous
"""Contrastive loss kernel for Trainium2 (8 NeuronCores).

Strategy: 2-D shard of the pairwise score computation (refining the
caption-axis sharding hint): 4-way split of the caption axis x 2-way split
of the image axis.  Each core computes its [16 captions * 50 words,
32 images * 36 regions] block of the raw pairwise dot tensor
g[i,w,j,r] = s[i,w,:] . im[j,r,:]  (the dominant 15 GFLOP contraction over
D=1024) on the TensorEngine in fp8-e4m3 with DoubleRow perf mode (2 fp8
weights per PE cell, 256-deep contraction per matmul) and fp32 PSUM
accumulation.  The 2-D shard minimizes per-core HBM traffic (s block +
im half ~ 2.0 MB vs 2.8 MB for a 1-D split).  g is written back as
fp8-e4m3 (|g| <= ~170 < 240 max).  The remaining cheap reductions
(leaky-relu attention, softmax, top-k word pooling, entity-matched direct
score, margin reduction) run on host in float32; fp8 quantization of the
g contraction shifts the final scalar by ~1e-3 relative, far inside the
2e-2 tolerance.
"""

import os
import sys

import numpy as np
import ml_dtypes

sys.path.insert(0, "/opt/trn_rl_repo")

B, R, L, D = 64, 36, 50, 1024
N_CORES = 8
CA, CB = 4, 2                  # caption-axis x image-axis core grid
BT_LOC = B // CA               # 16 captions per core
BI_LOC = B // CB               # 32 images per core
M = BT_LOC * L                 # 800 caption-words per core (moving dim)
MH = M // 2                    # 400-wide m-halves (PSUM free-dim limit)
N = BI_LOC * R                 # 1152 image-regions per core = 9 n-tiles
K = D                          # 1024 contraction
NT = N // 128                  # 9
KQ = K // 256                  # 4 DoubleRow k-pair tiles
LAMBDA_SOFTMAX = 9.0
MARGIN = 0.2
EPS = 1e-8

# im columns are DMA'd per k-pair in two chunks: tiles 0-3 and tiles 4-8
CHUNKS = [(0, 4), (4, 5)]      # (first tile, tile count)
# compute groups of <=2 n-tiles (4 PSUM banks each, 2 groups in flight)
GROUPS = [(0, 2), (2, 2), (4, 2), (6, 2), (8, 1)]
# output DMA groups (consecutive tiles shipped in one DMA) + lane each;
# the final tiles ship as singles on three different lanes so the closing
# chain never queues behind another out-DMA
OUT_GROUPS = [(0, 1), (2, 3), (4, 5), (6, 7), (8,)]
OUT_LANES = ["av", "sp", "av", "av", "sp"]
# psum->sbuf cast copies alternate between the DVE and Activation engines
COPY_ENGINES = ["dv", "av"]
FINAL_SPLIT = False

_CACHE = {}
LAST_RESULTS = None  # BassKernelResults from the most recent run (for test.py)

# Input DMA schedule: ordered ops, lane "sp" -> HWDGE via nc.sync,
# "pl" -> SWDGE on the Pool engine via nc.gpsimd.
# The host packs sT and imT row-wise into one DRAM tensor
# packed[k, :] = [sT[k, 0:800] | imT[k, 0:1152]], so one DMA fetches a
# k-pair's sT AND its first im chunk together (columns 0:1312):
#   ("pq", q, lane): packed rows of k-pair q, cols [0, 1312)  (st + im c0)
#   ("c1", q, lane): packed rows of k-pair q, cols [1312, 1952) (im c1)
SCHEDULE = [
    ("c0", 0, "sp"),
    ("st", 0, "sp"),
    ("c0", 1, "pl"),
    ("st", 1, "sp"),
    ("c0", 2, "pl"),
    ("st", 2, "sp"),
    ("c0", 3, "pl"),
    ("st", 3, "sp"),
    ("c1", 0, "sp"),
    ("c1", 1, "sp"),
    ("c1", 2, "sp"),
    ("c1a", 3, "sp"),
    ("c1b", 3, "sp"),
]
PQW = M + 4 * 128              # 1312 packed st+c0 columns
C0W = 4 * 128                  # 512 c0 columns (packed cols M:PQW)
C1W = 5 * 128                  # 640 c1 columns (packed cols PQW:)


def _build_bass():
    import concourse.bacc as bacc
    import concourse.mybir as mybir
    import concourse.tile as tile

    nc = bacc.Bacc(
        "TRN2",
        target_bir_lowering=False,
        debug=False,
        enable_asserts=False,
        num_devices=N_CORES,
    )
    f32 = mybir.dt.float32
    fp8 = mybir.dt.float8e4
    packed = nc.dram_tensor("packed", [K, M + N], fp8,
                            kind="ExternalInput").ap()
    gT = nc.dram_tensor("gT", [N, M], fp8, kind="ExternalOutput").ap()

    with tile.TileContext(nc) as tc:
        with (
            tc.tile_pool(name="pq", bufs=2 * KQ) as qp,
            tc.tile_pool(name="c1", bufs=KQ) as cp,
            tc.tile_pool(name="ps", bufs=8, space="PSUM") as pp,
            tc.tile_pool(name="out", bufs=4) as outp,
        ):
            # Two parallel descriptor-generation lanes: nc.sync -> HWDGE
            # (shared serial device, ~625ns/DMA) and nc.gpsimd -> SWDGE on
            # the otherwise-idle Pool engine (~1.1us/DMA).  Each k-pair's
            # sT + first im chunk land in ONE transfer (packed layout), so
            # a q-wave of matmuls unlocks atomically per arrival.
            lanes = {"sp": nc.sync, "pl": nc.gpsimd,
                     "dv": nc.vector, "av": nc.scalar}
            # per-q views: stv[q] -> (tile, base col); imv[(q, n-tile)] ->
            # (tile, base col).  Span kinds map to packed-column rectangles
            # and the n-tiles they carry (None = sT data).
            stv = [None] * KQ
            imv = {}
            spans = {"pq": (0, PQW, (0, 1, 2, 3)),
                     "st": (0, M, ()),
                     "c0": (M, C0W, (0, 1, 2, 3)),
                     "c1": (PQW, C1W, (4, 5, 6, 7, 8)),
                     "c1a": (PQW, 512, (4, 5, 6, 7)),
                     "c1b": (PQW + 512, 128, (8,))}
            for op in SCHEDULE:
                kind, q, lane = op
                col0, width, ntiles = spans[kind]
                pool = qp if kind in ("pq", "st", "c0") else cp
                t_ = pool.tile([128, 2 * width], fp8, tag=kind,
                               name=f"{kind}_{q}")
                lanes[lane].dma_start(
                    t_[:].rearrange("p (i x) -> p i x", i=2),
                    packed[q * 256:(q + 1) * 256,
                           col0:col0 + width].rearrange(
                        "(i p) x -> p i x", i=2),
                )
                if kind in ("pq", "st"):
                    stv[q] = (t_, 0)
                im0 = M if kind == "pq" else 0
                for k, t in enumerate(ntiles):
                    imv[(q, t)] = (t_, im0 + k * 128)

            def st_slice(q, h):
                t_, base = stv[q]
                return t_[:].rearrange("p (i x) -> p i x", i=2)[
                    :, :, base + h * MH:base + (h + 1) * MH]

            DR = mybir.MatmulPerfMode.DoubleRow

            def it_slice(q, t):
                t_, x0 = imv[(q, t)]
                return t_[:].rearrange("p (i x) -> p i x", i=2)[
                    :, :, x0:x0 + 128]

            tile_to_og = {}
            for ogi, og in enumerate(OUT_GROUPS):
                for t in og:
                    tile_to_og[t] = ogi
            og_tiles = {}   # ogi -> out sbuf tile

            def copy_ps(ce_name, dst, src):
                ce = lanes[ce_name]
                if ce is nc.scalar:
                    ce.copy(dst, src)
                else:
                    ce.tensor_copy(dst, src)

            for gi, (g0, gn) in enumerate(GROUPS):
                tiles = [(g0 + j, h) for j in range(gn) for h in range(2)]
                pss = {}
                for th in tiles:
                    ps = pp.tile([128, MH], f32, tag="ps", name=f"ps_{th[0]}_{th[1]}")
                    pss[th] = ps
                for q in range(KQ):
                    for (t, h) in tiles:
                        nc.tensor.matmul(
                            pss[(t, h)][:, :],
                            it_slice(q, t),
                            st_slice(q, h),
                            start=(q == 0),
                            stop=(q == KQ - 1),
                            perf_mode=DR,
                        )
                for j in range(gn):
                    t = g0 + j
                    ogi = tile_to_og[t]
                    og = OUT_GROUPS[ogi]
                    if ogi not in og_tiles:
                        og_tiles[ogi] = outp.tile(
                            [128, len(og) * M], fp8, tag="out",
                            name=f"out_{ogi}")
                    ot = og_tiles[ogi]
                    o0 = og.index(t) * M
                    if FINAL_SPLIT and t == NT - 1:
                        # ship each half right after its own copy so the
                        # closing DMA never waits for the sibling copy
                        for h in range(2):
                            copy_ps(COPY_ENGINES[(2 * t + h) % 2],
                                    ot[:, o0 + h * MH:o0 + (h + 1) * MH],
                                    pss[(t, h)][:, :])
                            lanes["av" if h == 0 else "sp"].dma_start(
                                gT[t * 128:(t + 1) * 128,
                                   h * MH:(h + 1) * MH],
                                ot[:, o0 + h * MH:o0 + (h + 1) * MH])
                        continue
                    for h in range(2):
                        copy_ps(COPY_ENGINES[(2 * t + h) % len(COPY_ENGINES)],
                                ot[:, o0 + h * MH:o0 + (h + 1) * MH],
                                pss[(t, h)][:, :])
                    if t == og[-1]:
                        ng = len(og)
                        dst = gT[og[0] * 128:(og[0] + ng) * 128, :]
                        if ng > 1:
                            dst = dst.rearrange("(b p) m -> p b m", b=ng)
                        lanes[OUT_LANES[ogi]].dma_start(dst, ot[:])
    nc.compile()
    return nc


def _run_device(s_np, im_np):
    """Returns g4 [B, B, L, R]: g4[i,j,w,r] = s[i,w] . im[j,r]."""
    global LAST_RESULTS
    from concourse import bass_utils

    if "nc" not in _CACHE:
        _CACHE["nc"] = _build_bass()
    nc = _CACHE["nc"]

    fp8 = ml_dtypes.float8_e4m3
    imq = im_np.reshape(B * R, K).astype(fp8)                   # [2304, 1024]
    sq = s_np.reshape(B * L, K).astype(fp8)                     # [3200, 1024]
    in_maps = []
    for c in range(N_CORES):
        ca, cb = divmod(c, CB)
        sblk = sq[ca * M:(ca + 1) * M]                          # [800, 1024]
        iblk = imq[cb * N:(cb + 1) * N]                         # [1152, 1024]
        pk = np.empty((K, M + N), dtype=fp8)
        pk[:, 0:M] = sblk.T
        pk[:, M:M + N] = iblk.T
        in_maps.append({"packed": pk})
    res = bass_utils.run_bass_kernel_spmd(
        nc, in_maps, core_ids=list(range(N_CORES)),
        trace=bool(os.environ.get("KERNEL_TRACE")),
    )
    LAST_RESULTS = res
    g4 = np.empty((B, B, L, R), dtype=np.float32)
    for c in range(N_CORES):
        ca, cb = divmod(c, CB)
        gb = np.asarray(res.results[c]["gT"], dtype=np.float32)  # [1152, 800]
        g4[ca * BT_LOC:(ca + 1) * BT_LOC,
           cb * BI_LOC:(cb + 1) * BI_LOC] = (
            gb.reshape(BI_LOC, R, BT_LOC, L).transpose(2, 0, 3, 1))
    return g4


def _host_finish(g4, im, s, img_ent, cap_ent, cap_lens):
    f32 = np.float32
    w_idx = np.arange(L)
    word_valid = w_idx[None, :] < cap_lens[:, None]             # [Bt, L]

    attn = np.where(g4 > 0, g4, f32(0.1) * g4)
    attn = attn * word_valid[:, None, :, None].astype(f32)
    attn = attn / (np.sqrt(np.sum(attn * attn, axis=2, keepdims=True)) + f32(EPS))
    z = attn * f32(LAMBDA_SOFTMAX)
    z = z - z.max(axis=-1, keepdims=True)
    e = np.exp(z)
    a = e / e.sum(axis=-1, keepdims=True)
    a = a * (a > 1.0 / R).astype(f32)

    dot_swc = np.sum(a * g4, axis=-1)                           # [Bt,Bi,L]
    gram = np.einsum("jrd,jqd->jrq", im, im)                    # [Bi,R,R]
    t = np.einsum("ijwr,jrq->ijwq", a, gram, optimize=True)
    wc_sq = np.sum(t * a, axis=-1)
    wc_norm = np.sqrt(np.maximum(wc_sq, f32(1e-24)))
    ns = np.sqrt(np.sum(s * s, axis=-1))                        # [Bt,L]
    cos = dot_swc / np.maximum(ns[:, None, :] * wc_norm, f32(EPS))
    cos = np.where(word_valid[:, None, :], cos, f32(-np.inf))
    srt = np.sort(cos, axis=-1)[..., ::-1]
    k = cap_lens - cap_lens // 3
    keep = w_idx[None, None, :] < k[:, None, None]
    latent = np.where(keep, srt, f32(0.0)).sum(axis=-1) / k[:, None].astype(f32)

    n_min = np.minimum(cap_lens, 50)
    ent_ok = (cap_ent != 0) & (w_idx[None, :] < n_min[:, None])
    match = (cap_ent[:, None, :, None] == img_ent[None, :, None, :]) \
        & ent_ok[:, None, :, None]
    nim = np.sqrt(np.sum(im * im, axis=-1))                     # [Bi,R]
    denom = np.maximum(ns[:, None, :, None] * nim[None, :, None, :], f32(EPS))
    direct = np.where(match, g4 / denom, f32(0.0)).sum(axis=(2, 3)) \
        / n_min[:, None].astype(f32)

    scores = latent + direct                                    # [Bt,Bi]
    diag = np.diag(scores).copy()
    cost_s = np.maximum(f32(MARGIN) + scores - diag[:, None], f32(0.0))
    cost_im = np.maximum(f32(MARGIN) + scores - diag[None, :], f32(0.0))
    np.fill_diagonal(cost_s, 0.0)
    np.fill_diagonal(cost_im, 0.0)
    return np.float32(cost_s.max(axis=1).sum() + cost_im.max(axis=0).sum())


def kernel(im, s, image_entity_idxs, caps_entity_idxs, s_l):
    im = np.asarray(im, dtype=np.float32)
    s = np.asarray(s, dtype=np.float32)
    img_ent = np.asarray(image_entity_idxs)
    cap_ent = np.asarray(caps_entity_idxs)
    cap_lens = np.asarray(s_l)
    g4 = _run_device(s, im)
    return _host_finish(g4, im, s, img_ent, cap_ent, cap_lens)
